# revision 1
# baseline (speedup 1.0000x reference)
"""Trainium2 Bass kernel for nn_Debias (histogram_binning).

Strategy (data-parallel over the sample dim, 8 cores):
  - Each core gets 125000 samples: pred [125000, 51] f32, gt [125000] i32.
  - Layout per core: 125 SBUF partitions x 1000 samples each, processed in
    8 chunks of 125 samples/partition (per-partition contiguous 25.5KB DMA).
  - Per chunk:
      rowmax   = reduce_max over classes 1..50              (DVE, segmented)
      oh_pred  = (pred[:,1:51] == rowmax)  -> bf16 one-hot  (DVE)
      oh_gt    = (gt == iota(51))          -> bf16 one-hot  (DVE)
      PSUM[50,51] += oh_pred_s^T @ oh_gt_s per sample column (PE, accumulate)
  - Row 0 of the confusion matrix is always 0 (argmax index is in [1,50]).
  - Host: sum the 8 local [51,51] histograms, then the small EMA postprocess.
"""

import numpy as np
from contextlib import ExitStack

from concourse import tile, bacc, mybir
from concourse.bass_utils import run_bass_kernel_spmd

N_CORES = 8
C = 51                 # num classes
NUM_SAMPLES = 1_000_000
S_CORE = NUM_SAMPLES // N_CORES   # 125000 samples per core
P = 128                # SBUF partitions (128 is ~2.2x faster DMA than <128)
SPP = 976              # samples per partition (main block: 128*976 = 124928)
F = 122                # samples per partition per chunk (even, for pairing)
NCHUNK = SPP // F      # 8 chunks
TAIL = S_CORE - P * SPP  # 72 leftover samples, one per partition column

f32 = mybir.dt.float32
bf16 = mybir.dt.bfloat16
i32 = mybir.dt.int32
i16 = mybir.dt.int16

_CACHE = {}


def _emit_histogram(nc, tc, ctx, pred_v, gt_v, tailp_v, tailg_v, hist_ap,
                    parts=("dma", "dve", "pe")):
    """Emit one full per-core histogram computation (all chunks + writeback).
    `parts` lets timing probes drop stages (data becomes garbage but the
    instruction mix/time of the remaining stages is preserved)."""
    const_pool = ctx.enter_context(tc.tile_pool(name="const", bufs=1))
    pred_pool = ctx.enter_context(tc.tile_pool(name="pred", bufs=3))
    gt_pool = ctx.enter_context(tc.tile_pool(name="gt", bufs=1))
    ohp_pool = ctx.enter_context(tc.tile_pool(name="ohp", bufs=3))
    ohg_pool = ctx.enter_context(tc.tile_pool(name="ohg", bufs=3))
    mx_pool = ctx.enter_context(tc.tile_pool(name="mx", bufs=3))
    out_pool = ctx.enter_context(tc.tile_pool(name="out", bufs=1))
    psum_pool = ctx.enter_context(tc.tile_pool(name="psum", bufs=1, space="PSUM"))

    # iota16rep[p, s, c] = c  (int16, repeated F times -> flat step-1 operand)
    iota_rep = const_pool.tile([P, F, C], i16)
    nc.gpsimd.iota(iota_rep[:], pattern=[[0, F], [1, C]], base=0,
                   channel_multiplier=0)
    gtrep_pool = ctx.enter_context(tc.tile_pool(name="gtrep", bufs=3))

    psum_t = psum_pool.tile([2 * (C - 1), 2 * C], f32)

    pred_flat = pred_v.rearrange("p s c -> p (s c)")
    gt_all = gt_pool.tile([P, SPP], i16)
    if "dma" in parts:
        nc.gpsimd.dma_start(gt_all[:], gt_v[:])
    else:
        nc.vector.memset(gt_all[:], 0)
    # tapered chunks: small first (faster pipeline fill) and small last
    # (shorter exposed PE/output drain); middle chunks full-size.
    SIZES = [30, 92] + [F] * 6 + [96, 26]
    assert sum(SIZES) == SPP
    offs = [sum(SIZES[:i]) for i in range(len(SIZES))]

    # gt one-hots depend only on the tiny gt DMA — emit them all up front so
    # the scheduler can fill any DVE stall (e.g. waiting on pred DMA) with
    # ohg work.
    ohgs = []
    for k, w in enumerate(SIZES):
        ohg = ohg_pool.tile([P, w, C], bf16, tag="ohg")
        if "dve" in parts:
            gtrep = gtrep_pool.tile([P, w, C], i16, tag="gtrep")
            nc.scalar.copy(gtrep[:],
                           gt_all[:, offs[k]:offs[k] + w]
                           .unsqueeze(2).broadcast_to([P, w, C]))
            nc.vector.tensor_tensor(
                ohg[:], gtrep[:], iota_rep[:, 0:w, :],
                op=mybir.AluOpType.is_equal)
        elif "pe" in parts:
            nc.vector.memset(ohg[:], 0.0)
        ohgs.append(ohg)

    for k, w in enumerate(SIZES):
        off = offs[k]
        predt = pred_pool.tile([P, w, C], f32, tag="predt")
        if "dma" in parts:
            eng = nc.sync if k % 2 == 0 else nc.scalar
            eng.dma_start(predt[:].rearrange("p s c -> p (s c)"),
                          pred_flat[:, off * C:(off + w) * C])

        if "dma" not in parts:
            # timing probes: producers on ACT (no DVE port contention)
            nc.scalar.memzero(predt[:].rearrange("p s c -> p (s c)"))

        mxt = mx_pool.tile([P, w], f32, tag="mxt")
        ohp = ohp_pool.tile([P, w, C - 1], bf16, tag="ohp")
        ohg = ohgs[k]
        if "dve" not in parts and "pe" in parts:
            nc.vector.memset(ohp[:], 0.0)
        if "dve" in parts:
            nc.vector.tensor_reduce(
                mxt[:], predt[:, :, 1:C],
                axis=mybir.AxisListType.X, op=mybir.AluOpType.max)
            nc.vector.tensor_tensor(
                ohp[:], predt[:, :, 1:C],
                mxt[:].unsqueeze(2).broadcast_to([P, w, C - 1]),
                op=mybir.AluOpType.is_equal)

        if "pe" in parts:
            for s in range(0, w, 2):
                # two samples fused: lhsT [P, 2*(C-1)], rhs [P, 2*C];
                # useful results live in the two diagonal PSUM blocks.
                nc.tensor.matmul(
                    psum_t[:],
                    lhsT=ohp[:, s:s + 2, :].rearrange("p s c -> p (s c)"),
                    rhs=ohgs[k][:, s:s + 2, :].rearrange("p s c -> p (s c)"),
                    start=(k == 0 and s == 0),
                    stop=False)

    # --- tail: 72 leftover samples, one per partition (single matmul, K=TAIL)
    predt_t = pred_pool.tile([TAIL, 1, C], f32, tag="predtail")
    gtt_t = gt_pool.tile([TAIL, 1], i16, tag="gttail")
    mxt_t = mx_pool.tile([TAIL, 1], f32, tag="mxtail")
    ohp_t = ohp_pool.tile([TAIL, 1, C - 1], bf16, tag="ohptail")
    ohg_t = ohg_pool.tile([TAIL, 1, C], bf16, tag="ohgtail")
    if "dma" in parts:
        nc.sync.dma_start(predt_t[:].rearrange("p s c -> p (s c)"),
                          tailp_v[:].rearrange("p s c -> p (s c)"))
        nc.gpsimd.dma_start(gtt_t[:], tailg_v[:])
    else:
        nc.scalar.memzero(predt_t[:].rearrange("p s c -> p (s c)"))
        nc.vector.memset(gtt_t[:], 0)
    if "dve" in parts:
        gtrep_t = gtrep_pool.tile([TAIL, 1, C], i16, tag="gtreptail")
        nc.scalar.copy(gtrep_t[:],
                       gtt_t[:].unsqueeze(2).broadcast_to([TAIL, 1, C]))
        nc.vector.tensor_tensor(
            ohg_t[:], gtrep_t[:], iota_rep[0:TAIL, 0, :].unsqueeze(1),
            op=mybir.AluOpType.is_equal)
        nc.vector.tensor_reduce(
            mxt_t[:], predt_t[:, :, 1:C],
            axis=mybir.AxisListType.X, op=mybir.AluOpType.max)
        nc.vector.tensor_tensor(
            ohp_t[:], predt_t[:, :, 1:C],
            mxt_t[:].unsqueeze(2).broadcast_to([TAIL, 1, C - 1]),
            op=mybir.AluOpType.is_equal)
    elif "pe" in parts:
        nc.vector.memset(ohp_t[:], 0.0)
        nc.vector.memset(ohg_t[:], 0.0)
    if "pe" in parts:
        nc.tensor.matmul(
            psum_t[0:C - 1, 0:C],
            lhsT=ohp_t[:, 0, :], rhs=ohg_t[:, 0, :],
            start=False, stop=True)

    histb = out_pool.tile([2 * (C - 1), 2 * C], f32)
    if "pe" not in parts:
        nc.vector.memset(psum_t[:], 0.0)
    nc.scalar.copy(histb[:], psum_t[:])
    nc.sync.dma_start(hist_ap[:], histb[:])


def _build(repeat=None, internal_io=False, parts=("dma", "dve", "pe")):
    """repeat=None: production build (external pred/gt).
    repeat=R with internal_io=True: timing build — pred/gt are internal DRAM
    scratch (no host transfer), whole computation looped R times in-NEFF."""
    nc = bacc.Bacc("TRN2", target_bir_lowering=False, debug=False,
                   num_devices=N_CORES)
    if internal_io:
        dummy_ap = nc.dram_tensor("tick", [1], f32, kind="ExternalInput").ap()
        pred_ap = nc.dram_tensor("pred_i", [S_CORE, C], f32).ap()
        gt_ap = nc.dram_tensor("gt_i", [S_CORE], i16).ap()
    else:
        pred_ap = nc.dram_tensor("pred", [S_CORE, C], f32,
                                 kind="ExternalInput").ap()
        gt_ap = nc.dram_tensor("gt", [S_CORE], i16, kind="ExternalInput").ap()
    hist_ap = nc.dram_tensor("hist", [2 * (C - 1), 2 * C], f32,
                             kind="ExternalOutput").ap()

    main = P * SPP
    pred_v = pred_ap[0:main].rearrange("(p s) c -> p s c", p=P)
    gt_v = gt_ap[0:main].rearrange("(p s) -> p s", p=P)
    tailp_v = pred_ap[main:S_CORE].rearrange("(p s) c -> p s c", p=TAIL)
    tailg_v = gt_ap[main:S_CORE].rearrange("(p s) -> p s", p=TAIL)

    with tile.TileContext(nc) as tc:
        with ExitStack() as ctx:
            if repeat is None:
                _emit_histogram(nc, tc, ctx, pred_v, gt_v, tailp_v, tailg_v, hist_ap, parts=parts)
            else:
                with tc.For_i(0, repeat, 1,
                              hint_engines=(mybir.EngineType.PE,
                                            mybir.EngineType.DVE)):
                    _emit_histogram(nc, tc, ctx, pred_v, gt_v, tailp_v, tailg_v, hist_ap, parts=parts)
    nc.compile()
    return nc


def _get_nc():
    if "nc" not in _CACHE:
        _CACHE["nc"] = _build()
    return _CACHE["nc"]


def _device_histogram(pred: np.ndarray, gt: np.ndarray,
                      want_trace: bool = False):
    """Run the SPMD kernel; return (global [51,51] f32 histogram, results)."""
    nc = _get_nc()
    pred = np.ascontiguousarray(pred, dtype=np.float32)
    gt = np.ascontiguousarray(gt, dtype=np.int16)
    in_maps = [
        {"pred": pred[i * S_CORE:(i + 1) * S_CORE],
         "gt": gt[i * S_CORE:(i + 1) * S_CORE]}
        for i in range(N_CORES)
    ]
    res = run_bass_kernel_spmd(nc, in_maps, list(range(N_CORES)),
                               trace=want_trace)
    hist = np.zeros((C, C), dtype=np.float32)
    for r in res.results:
        hb = r["hist"]
        # diagonal blocks: [0:50, 0:51] (even samples) + [50:100, 51:102] (odd)
        hist[1:C, :] += hb[0:C - 1, 0:C] + hb[C - 1:2 * (C - 1), C:2 * C]
    return hist, res


def kernel(pred, rel_count, gt, istrain):
    pred = np.asarray(pred)
    rel_count = np.asarray(rel_count, dtype=np.float32)
    if not int(np.asarray(istrain)):
        return rel_count

    num = pred.shape[0]
    hist, _ = _device_histogram(pred, np.asarray(gt))

    # Small [51,51] postprocessing (exact mirror of the reference, f32).
    idx = hist.sum(axis=1, dtype=np.float32) / np.float32(num)
    gate = np.where(idx > 0.0, np.float32(0.9), np.float32(1.0))
    hist = hist.copy()
    hist[:, 0] = 0.0
    norm = hist / (hist.sum(axis=1, keepdims=True, dtype=np.float32)
                   + np.float32(1e-10))
    norm = norm.astype(np.float32)
    ema = gate[:, None] * rel_count + np.float32(0.1) * norm
    out = np.where(rel_count.sum(dtype=np.float32) == 0.0, norm, ema)
    return out.astype(np.float32)



# revision 2
# speedup vs baseline: 1.0091x; 1.0091x over previous
"""Trainium2 Bass kernel v6 for nn_Debias (histogram_binning).

Per core: 125000 samples; device handles 124928 as [128 x 976], the 72
leftover samples are folded in on the host (0.006% of work, f32-exact).

Pipeline per chunk of w samples/partition (w in SIZES):
  - gpsimd SWDGE cast-DMA: pred f32 (HBM) -> bf16 SBUF, ~339 GB/s.
  - DVE m1 = max(pred[:,1:26], pred[:,26:51]); reduce25 -> mxt [P,w].
  - ACT duplicates mxt -> mxt2 [P,w,2] and gt -> gt2 [P,w,2].
  - pair-flat compare (the key trick): two samples' classes are read as
    overlapping 52-wide contiguous runs via a raw 5-D AP
    [p][102,g][51,s][2,a][1,b], compared against the mxt2 pair-broadcast
    [p][g][s][0,a][1,b].  Every operand keeps a packed innermost dim, so
    the DVE 16-bit 2x mode stays on (a plain stride-0 broadcast or a
    51-strided slice would drop it).  Output ohp52 [P,w,52]; cols 0 and
    51 are garbage (class 0 / next sample's class 0) and the host drops
    the corresponding PSUM rows.
  - ohg52 [P,w,52] = is_equal(gt2 pair-broadcast, iota52), same trick.
  - PE: per 2 samples, lhsT = ohp52 pair [P,104], rhs = ohg52 pair
    [P,104] accumulate into PSUM [104,104].
Numerics: bf16 argmax ties double-fire ~1.2% of samples; end-to-end
l2 rel error vs the f32 reference is ~2e-5 (gate 2e-2).
"""

import numpy as np
from contextlib import ExitStack

from concourse import tile, bacc, mybir
from concourse.ap import AP
from concourse.bass_utils import run_bass_kernel_spmd

N_CORES = 8
C = 51
C2 = 52
NUM_SAMPLES = 1_000_000
S_CORE = NUM_SAMPLES // N_CORES
P = 128
SPP = 976
MAIN = P * SPP                # 124928 device samples per core
TAIL = S_CORE - MAIN          # 72, host side

f32 = mybir.dt.float32
bf16 = mybir.dt.bfloat16
i16 = mybir.dt.int16

_CACHE = {}

SIZES = [122] * 8
assert sum(SIZES) == SPP and all(w % 2 == 0 for w in SIZES)
MAXW = max(SIZES)


def _emit(nc, tc, ctx, pred_v, gt_v, hist_ap, parts=("dma", "dve", "pe")):
    const_pool = ctx.enter_context(tc.tile_pool(name="const", bufs=1))
    pred_pool = ctx.enter_context(tc.tile_pool(name="pred", bufs=4))
    m1_pool = ctx.enter_context(tc.tile_pool(name="m1", bufs=2))
    gt_pool = ctx.enter_context(tc.tile_pool(name="gt", bufs=1))
    ohp_pool = ctx.enter_context(tc.tile_pool(name="ohp", bufs=3))
    ohg_pool = ctx.enter_context(tc.tile_pool(name="ohg", bufs=3))
    mx_pool = ctx.enter_context(tc.tile_pool(name="mx", bufs=2))
    dup_pool = ctx.enter_context(tc.tile_pool(name="dup", bufs=2))
    gt2_pool = ctx.enter_context(tc.tile_pool(name="gt2", bufs=1))
    out_pool = ctx.enter_context(tc.tile_pool(name="out", bufs=1))
    psum_pool = ctx.enter_context(tc.tile_pool(name="psum", bufs=1, space="PSUM"))

    iota52 = const_pool.tile([P, MAXW, C2], i16)
    nc.gpsimd.iota(iota52[:], pattern=[[0, MAXW], [1, C2]], base=0,
                   channel_multiplier=0)

    psum_t = psum_pool.tile([2 * C2, 2 * C2], f32)

    pred_flat = pred_v.rearrange("p s c -> p (s c)")
    gt_all = gt_pool.tile([P, SPP], i16)
    if "dma" in parts:
        nc.sync.dma_start(gt_all[:], gt_v[:])
    else:
        nc.vector.memset(gt_all[:], 0)

    offs = [sum(SIZES[:i]) for i in range(len(SIZES))]

    # gt2 dups for every chunk up front on ACT (only needs gt_all)
    gt2s = []
    for k, w in enumerate(SIZES):
        gt2 = gt2_pool.tile([P, w, 2], i16, tag=f"gt2_{k}")
        if "dve" in parts:
            nc.scalar.copy(gt2[:],
                           gt_all[:, offs[k]:offs[k] + w]
                           .unsqueeze(2).broadcast_to([P, w, 2]))
        else:
            nc.vector.memset(gt2[:], 0)
        gt2s.append(gt2)

    for k, w in enumerate(SIZES):
        off = offs[k]
        L = w * C
        predf = pred_pool.tile([P, L + 2], bf16, tag="predf")
        if "dma" in parts:
            nc.gpsimd.dma_start(predf[:, 0:L],
                                pred_flat[:, off * C:(off + w) * C])
        else:
            nc.gpsimd.memset(predf[:, 0:L], 0)
        pred3 = predf[:, 0:L].rearrange("p (w c) -> p w c", c=C)

        ohg = ohg_pool.tile([P, w, C2], bf16, tag="ohg")
        if "dve" in parts:
            nc.vector.tensor_tensor(
                ohg[:].rearrange("p w (a b) -> p w a b", b=2),
                gt2s[k][:].unsqueeze(2).broadcast_to([P, w, C2 // 2, 2]),
                iota52[:, 0:w, :].rearrange("p w (a b) -> p w a b", b=2),
                op=mybir.AluOpType.is_equal)
        elif "pe" in parts:
            nc.vector.memset(ohg[:], 0.0)

        mxt = mx_pool.tile([P, w], bf16, tag="mxt")
        mxt2 = dup_pool.tile([P, w, 2], bf16, tag="mxt2")
        ohp = ohp_pool.tile([P, w, C2], bf16, tag="ohp")
        if "dve" not in parts and "pe" in parts:
            nc.vector.memset(ohp[:], 0.0)
        if "dve" in parts:
            m1 = m1_pool.tile([P, w, 25], bf16, tag="m1")
            nc.vector.tensor_tensor(
                m1[:], pred3[:, :, 1:26], pred3[:, :, 26:C],
                op=mybir.AluOpType.max)
            nc.vector.tensor_reduce(
                mxt[:], m1[:],
                axis=mybir.AxisListType.X, op=mybir.AluOpType.max)
            nc.scalar.copy(mxt2[:],
                           mxt[:].unsqueeze(2).broadcast_to([P, w, 2]))
            # pair-flat compare: overlapping 52-wide contiguous reads
            base = predf[:]
            in0 = AP(base.tensor, base.offset,
                     [list(base.ap[0]), [2 * C, w // 2], [C, 2], [2, 26],
                      [1, 2]])
            in1 = (mxt2[:].rearrange("p (g s) b -> p g s b", s=2)
                   .unsqueeze(3).broadcast_to([P, w // 2, 2, 26, 2]))
            out5 = ohp[:].rearrange("p (g s) (a b) -> p g s a b", s=2, b=2)
            nc.vector.tensor_tensor(out5, in0, in1,
                                    op=mybir.AluOpType.is_equal)

        if "pe" in parts:
            for s in range(0, w, 2):
                nc.tensor.matmul(
                    psum_t[:],
                    lhsT=ohp[:, s:s + 2, :].rearrange("p s c -> p (s c)"),
                    rhs=ohg[:, s:s + 2, :].rearrange("p s c -> p (s c)"),
                    start=(k == 0 and s == 0),
                    stop=(k == len(SIZES) - 1 and s == w - 2))

    histb = out_pool.tile([2 * C2, 2 * C2], f32)
    if "pe" not in parts:
        nc.vector.memset(psum_t[:], 0.0)
    nc.scalar.copy(histb[:], psum_t[:])
    nc.sync.dma_start(hist_ap[:], histb[:])


def _build(repeat=None, internal_io=False, parts=("dma", "dve", "pe")):
    nc = bacc.Bacc("TRN2", target_bir_lowering=False, debug=False,
                   num_devices=N_CORES)
    if internal_io:
        nc.dram_tensor("tick", [1], f32, kind="ExternalInput").ap()
        pred_ap = nc.dram_tensor("pred_i", [MAIN, C], f32).ap()
        gt_ap = nc.dram_tensor("gt_i", [MAIN], i16).ap()
    else:
        pred_ap = nc.dram_tensor("pred", [MAIN, C], f32,
                                 kind="ExternalInput").ap()
        gt_ap = nc.dram_tensor("gt", [MAIN], i16, kind="ExternalInput").ap()
    hist_ap = nc.dram_tensor("hist", [2 * C2, 2 * C2], f32,
                             kind="ExternalOutput").ap()

    pred_v = pred_ap.rearrange("(p s) c -> p s c", p=P)
    gt_v = gt_ap.rearrange("(p s) -> p s", p=P)

    with tile.TileContext(nc) as tc:
        with ExitStack() as ctx:
            if repeat is None:
                _emit(nc, tc, ctx, pred_v, gt_v, hist_ap, parts=parts)
            else:
                with tc.For_i(0, repeat, 1,
                              hint_engines=(mybir.EngineType.PE,
                                            mybir.EngineType.DVE)):
                    _emit(nc, tc, ctx, pred_v, gt_v, hist_ap, parts=parts)
    nc.compile()
    return nc


def _get_nc():
    if "nc" not in _CACHE:
        _CACHE["nc"] = _build()
    return _CACHE["nc"]


def _device_histogram(pred: np.ndarray, gt: np.ndarray):
    nc = _get_nc()
    in_maps = [
        {"pred": np.ascontiguousarray(
            pred[i * S_CORE:i * S_CORE + MAIN], dtype=np.float32),
         "gt": np.ascontiguousarray(
             gt[i * S_CORE:i * S_CORE + MAIN], dtype=np.int16)}
        for i in range(N_CORES)
    ]
    res = run_bass_kernel_spmd(nc, in_maps, list(range(N_CORES)))
    hist = np.zeros((C, C), dtype=np.float32)
    for r in res.results:
        hb = r["hist"]
        # even-sample block rows 1..50; odd block rows 53..102 (garbage
        # rows 0/51/52/103 dropped), gt cols 0..50 in each block
        hist[1:C, :] += hb[1:C, 0:C] + hb[C2 + 1:C2 + C, C2:C2 + C]
    return hist


def kernel(pred, rel_count, gt, istrain):
    pred = np.asarray(pred)
    rel_count = np.asarray(rel_count, dtype=np.float32)
    if not int(np.asarray(istrain)):
        return rel_count

    num = pred.shape[0]
    gt_np = np.asarray(gt)
    hist = _device_histogram(pred, gt_np)

    # host-side tail: last 72 samples of each core's shard, exact f32
    for i in range(N_CORES):
        lo, hi = i * S_CORE + MAIN, (i + 1) * S_CORE
        tp = np.asarray(pred[lo:hi], dtype=np.float32)
        tg = np.asarray(gt_np[lo:hi], dtype=np.int64)
        ti = tp[:, 1:].argmax(axis=1) + 1
        np.add.at(hist, (ti, tg), np.float32(1.0))

    idx = hist.sum(axis=1, dtype=np.float32) / np.float32(num)
    gate = np.where(idx > 0.0, np.float32(0.9), np.float32(1.0))
    hist = hist.copy()
    hist[:, 0] = 0.0
    norm = hist / (hist.sum(axis=1, keepdims=True, dtype=np.float32)
                   + np.float32(1e-10))
    norm = norm.astype(np.float32)
    ema = gate[:, None] * rel_count + np.float32(0.1) * norm
    out = np.where(rel_count.sum(dtype=np.float32) == 0.0, norm, ema)
    return out.astype(np.float32)
